# revision 14
# baseline (speedup 1.0000x reference)
"""nn_DegreeDeviation — TRN2 Bass kernel (8 NeuronCores, SPMD).

kernel(**inputs) takes the FULL inputs from reference.setup_inputs() and
returns the FULL [num_nodes] float32 output.

Strategy (per sharding hint): shard the 64M edge endpoints evenly across the
8 cores; each core builds a local 1,048,576-bin histogram with the one-hot
outer-product matmul trick (PSUM-accumulated); AllReduce the [128, 8192] f32
degree grid; every core normalizes redundantly; host reads core 0's output.
"""

import sys

sys.path.insert(0, "/opt/trn_rl_repo")

from contextlib import ExitStack

import numpy as np

import concourse.bass as bass
import concourse.tile as tile
from concourse import bacc, mybir
from concourse.bass import ds, ts
from concourse.bass_utils import run_bass_kernel_spmd

P = 128
LO = 8192          # lo bins per partition row
HALF = 4096        # PSUM-resident half of the lo range
NUM_NODES = 1_000_000
NUM_EDGES = 32_000_000
NUM_BINS = P * LO  # 1,048,576 padded bins
PAD_BIN = NUM_BINS - 1
N_CORES = 8

TILES = 123        # per-core input tiles of [128, COLS]
COLS = 512
GROUP_UNROLL = 32

f32 = mybir.dt.float32
bf16 = mybir.dt.bfloat16
i32 = mybir.dt.int32
i16 = mybir.dt.int16
Alu = mybir.AluOpType

_CACHED_NC = None


def build_kernel(tiles: int = TILES, cols: int = COLS,
                 group_unroll: int = GROUP_UNROLL, n_cores: int = N_CORES):
    assert cols % group_unroll == 0 and cols >= 2 * group_unroll
    nc = bacc.Bacc("TRN2", target_bir_lowering=False, debug=False,
                   num_devices=n_cores)

    edges = nc.dram_tensor("edges", [tiles * P, cols], i32, kind="ExternalInput")
    out_d = nc.dram_tensor("out", [P, LO], f32, kind="ExternalOutput")
    cc_in = nc.dram_tensor("cc_in", [P, LO], f32)
    cc_out = nc.dram_tensor("cc_out", [P, LO], f32, addr_space="Shared")

    with tile.TileContext(nc) as tc, ExitStack() as ctx:
        const_pool = ctx.enter_context(tc.tile_pool(name="const", bufs=1))
        hist_pool = ctx.enter_context(tc.tile_pool(name="hist", bufs=1))
        in_pool = ctx.enter_context(tc.tile_pool(name="inp", bufs=2))
        ext_pool = ctx.enter_context(tc.tile_pool(name="ext", bufs=2))
        oh_pool = ctx.enter_context(tc.tile_pool(name="oh", bufs=3))
        hioh_pool = ctx.enter_context(tc.tile_pool(name="hioh", bufs=4))
        psum_pool = ctx.enter_context(tc.tile_pool(name="psum", bufs=1, space="PSUM"))
        stat_pool = ctx.enter_context(tc.tile_pool(name="stat", bufs=1))
        sq_pool = ctx.enter_context(tc.tile_pool(name="sq", bufs=1))

        B = 2  # groups per one-hot build instruction

        # --- constants ---
        # iota_rep: values 0..HALF-1 repeated B times along the free dim
        iota_rep = const_pool.tile([P, B * HALF], i16, tag="iota_rep")
        nc.gpsimd.iota(iota_rep[:].rearrange("p (b f) -> p b f", b=B),
                       [[0, B], [1, HALF]], channel_multiplier=0)
        iota_hi_rep = const_pool.tile([P, B * P], i16, tag="iota_hi_rep")
        nc.gpsimd.iota(iota_hi_rep[:].rearrange("p (b f) -> p b f", b=B),
                       [[0, B], [1, P]], channel_multiplier=0)
        ones_col = const_pool.tile([P, 1], f32, tag="ones_col")
        nc.vector.memset(ones_col[:], 1.0)
        ones_row = const_pool.tile([1, P], f32, tag="ones_row")
        nc.vector.memset(ones_row[:], 1.0)

        # validity mask: 1.0 where global bin index p*LO + f < NUM_NODES
        row_base = const_pool.tile([P, 1], f32, tag="row_base")
        nc.gpsimd.iota(row_base[:], [[1, 1]], channel_multiplier=LO,
                       allow_small_or_imprecise_dtypes=True)
        row_base2 = const_pool.tile([P, 1], f32, tag="row_base2")
        nc.vector.tensor_scalar(out=row_base2[:], in0=row_base[:],
                                scalar1=float(HALF), scalar2=None, op0=Alu.add)
        mask = const_pool.tile([P, LO], f32, tag="mask")
        for h, rb in ((0, row_base), (1, row_base2)):
            sl = mask[:, h * HALF:(h + 1) * HALF]
            nc.vector.tensor_copy(out=sl, in_=iota_rep[:, :HALF])
            nc.vector.tensor_scalar(out=sl, in0=sl, scalar1=rb[:, :1],
                                    scalar2=None, op0=Alu.add)
            nc.vector.tensor_scalar(out=sl, in0=sl, scalar1=float(NUM_NODES),
                                    scalar2=None, op0=Alu.is_lt)

        hist = hist_pool.tile([P, LO], f32, tag="hist")
        nc.vector.memset(hist[:], 0)

        psum = psum_pool.tile([P, HALF], f32, tag="ps")

        G = group_unroll
        assert G % B == 0

        def build_onehots(loh, hi16, col):
            """One-hot tiles for B consecutive groups starting at `col`.

            loh holds lo - h*HALF, so comparing against iota 0..HALF-1
            selects exactly the current half's elements."""
            oh = oh_pool.tile([P, B * HALF], bf16, tag="oh")
            nc.vector.tensor_tensor(
                out=oh[:].rearrange("p (b f) -> p b f", b=B),
                in0=loh[:, ds(col, B)].to_broadcast([P, B, HALF]),
                in1=iota_rep[:].rearrange("p (b f) -> p b f", b=B),
                op=Alu.is_equal,
            )
            hioh = hioh_pool.tile([P, B * P], bf16, tag="hioh")
            nc.vector.tensor_tensor(
                out=hioh[:].rearrange("p (b f) -> p b f", b=B),
                in0=hi16[:, ds(col, B)].to_broadcast([P, B, P]),
                in1=iota_hi_rep[:].rearrange("p (b f) -> p b f", b=B),
                op=Alu.is_equal,
            )
            return oh, hioh

        def matmul_batch(oh, hioh, start):
            for u in range(B):
                for b in range(HALF // 512):
                    nc.tensor.matmul(
                        out=psum[:, b * 512:(b + 1) * 512],
                        lhsT=hioh[:, u * P:(u + 1) * P],
                        rhs=oh[:, u * HALF + b * 512:u * HALF + (b + 1) * 512],
                        start=start and u == 0, stop=False,
                        skip_group_check=True,
                    )

        # --- histogram ---
        with tc.For_i(0, tiles, staggered_reset=True) as t:
            tl = in_pool.tile([P, cols], i32, tag="tl")
            nc.sync.dma_start(out=tl[:], in_=edges[ts(t, P), :])

            lo32 = ext_pool.tile([P, cols], i32, tag="lo32")
            nc.vector.tensor_scalar(out=lo32[:], in0=tl[:], scalar1=LO - 1,
                                    scalar2=None, op0=Alu.bitwise_and)
            hi32 = ext_pool.tile([P, cols], i32, tag="hi32")
            nc.vector.tensor_scalar(out=hi32[:], in0=tl[:], scalar1=13,
                                    scalar2=None, op0=Alu.logical_shift_right)
            lo16 = ext_pool.tile([P, cols], i16, tag="lo16")
            nc.vector.tensor_copy(out=lo16[:], in_=lo32[:])
            hi16 = ext_pool.tile([P, cols], i16, tag="hi16")
            nc.vector.tensor_copy(out=hi16[:], in_=hi32[:])
            # lo shifted into the second half's window (h=1 pass)
            lo16b = ext_pool.tile([P, cols], i16, tag="lo16b")
            nc.vector.tensor_scalar(out=lo16b[:], in0=lo16[:],
                                    scalar1=HALF, scalar2=None,
                                    op0=Alu.subtract)

            for h in range(2):
                loh = lo16 if h == 0 else lo16b
                oh0, hioh0 = build_onehots(loh, hi16, 0)
                matmul_batch(oh0, hioh0, start=True)
                for w in range(B, G, B):
                    ohw, hiohw = build_onehots(loh, hi16, w)
                    matmul_batch(ohw, hiohw, start=False)
                with tc.For_i(G, cols, G, name=f"grp_h{h}", staggered_reset=True) as j:
                    for w in range(0, G, B):
                        ohj, hiohj = build_onehots(loh, hi16, j + w)
                        matmul_batch(ohj, hiohj, start=False)
                nc.vector.tensor_add(
                    out=hist[:, h * HALF:(h + 1) * HALF],
                    in0=hist[:, h * HALF:(h + 1) * HALF],
                    in1=psum[:],
                )

        # --- AllReduce across cores ---
        cc_sem = nc.alloc_semaphore("cc_sem")
        dma_sem = nc.alloc_semaphore("cc_dma_sem")
        with tc.tile_critical():
            nc.sync.dma_start(out=cc_in[:], in_=hist[:]).then_inc(dma_sem, 16)
            nc.gpsimd.wait_ge(dma_sem, 16)
            nc.gpsimd.collective_compute(
                "AllReduce", Alu.add,
                replica_groups=[list(range(n_cores))],
                ins=[cc_in[:]], outs=[cc_out[:]],
            ).then_inc(cc_sem)
            nc.sync.wait_ge(cc_sem, 1)
            nc.sync.dma_start(out=hist[:], in_=cc_out[:]).then_inc(dma_sem, 16)
            nc.sync.wait_ge(dma_sem, 32)

        # --- zero padded bins ---
        nc.vector.tensor_tensor(out=hist[:], in0=hist[:], in1=mask[:], op=Alu.mult)

        # --- mean ---
        rowsum = stat_pool.tile([P, 1], f32, tag="rowsum")
        nc.vector.tensor_reduce(out=rowsum[:], in_=hist[:],
                                axis=mybir.AxisListType.X, op=Alu.add)
        tot_ps = psum_pool.tile([1, 1], f32, tag="ps")
        nc.tensor.matmul(out=tot_ps[:], lhsT=rowsum[:], rhs=ones_col[:],
                         start=True, stop=True, skip_group_check=True)
        mean = stat_pool.tile([1, 1], f32, tag="mean")
        nc.vector.tensor_scalar(out=mean[:], in0=tot_ps[:],
                                scalar1=1.0 / NUM_NODES, scalar2=None,
                                op0=Alu.mult)
        mean_bc_ps = psum_pool.tile([P, 1], f32, tag="ps")
        nc.tensor.matmul(out=mean_bc_ps[:], lhsT=ones_row[:], rhs=mean[:],
                         start=True, stop=True, skip_group_check=True)
        mean_bc = stat_pool.tile([P, 1], f32, tag="mean_bc")
        nc.vector.tensor_copy(out=mean_bc[:], in_=mean_bc_ps[:])

        # centered = (hist - mean) * mask
        nc.vector.tensor_scalar(out=hist[:], in0=hist[:],
                                scalar1=mean_bc[:, :1], scalar2=None,
                                op0=Alu.subtract)
        nc.vector.tensor_tensor(out=hist[:], in0=hist[:], in1=mask[:], op=Alu.mult)

        # ss = sum(centered^2)
        sqsum = stat_pool.tile([P, 1], f32, tag="sqsum")
        for h in range(2):
            sq = sq_pool.tile([P, HALF], f32, tag="sq")
            nc.vector.tensor_tensor(out=sq[:],
                                    in0=hist[:, h * HALF:(h + 1) * HALF],
                                    in1=hist[:, h * HALF:(h + 1) * HALF],
                                    op=Alu.mult)
            half_sum = stat_pool.tile([P, 1], f32, tag=f"half_sum{h}")
            nc.vector.tensor_reduce(out=half_sum[:], in_=sq[:],
                                    axis=mybir.AxisListType.X, op=Alu.add)
            if h == 0:
                nc.vector.tensor_copy(out=sqsum[:], in_=half_sum[:])
            else:
                nc.vector.tensor_add(out=sqsum[:], in0=sqsum[:], in1=half_sum[:])

        ss_ps = psum_pool.tile([1, 1], f32, tag="ps")
        nc.tensor.matmul(out=ss_ps[:], lhsT=sqsum[:], rhs=ones_col[:],
                         start=True, stop=True, skip_group_check=True)
        var = stat_pool.tile([1, 1], f32, tag="var")
        nc.vector.tensor_scalar(out=var[:], in0=ss_ps[:],
                                scalar1=1.0 / (NUM_NODES - 1), scalar2=None,
                                op0=Alu.mult)
        std = stat_pool.tile([1, 1], f32, tag="std")
        nc.scalar.sqrt(out=std[:], in_=var[:])
        nc.vector.tensor_scalar(out=std[:], in0=std[:], scalar1=1e-8,
                                scalar2=None, op0=Alu.add)
        inv = stat_pool.tile([1, 1], f32, tag="inv")
        nc.vector.reciprocal(out=inv[:], in_=std[:])
        inv_bc_ps = psum_pool.tile([P, 1], f32, tag="ps")
        nc.tensor.matmul(out=inv_bc_ps[:], lhsT=ones_row[:], rhs=inv[:],
                         start=True, stop=True, skip_group_check=True)
        inv_bc = stat_pool.tile([P, 1], f32, tag="inv_bc")
        nc.vector.tensor_copy(out=inv_bc[:], in_=inv_bc_ps[:])

        nc.vector.tensor_scalar(out=hist[:], in0=hist[:],
                                scalar1=inv_bc[:, :1], scalar2=None,
                                op0=Alu.mult)
        nc.sync.dma_start(out=out_d[:], in_=hist[:])

    nc.compile()
    return nc


def shard_inputs(edge_index: np.ndarray, tiles: int = TILES, cols: int = COLS,
                 n_cores: int = N_CORES):
    flat = np.ascontiguousarray(edge_index, dtype=np.int32).reshape(-1)
    n = flat.shape[0]
    assert n % n_cores == 0
    per = n // n_cores
    cap = tiles * P * cols
    assert cap >= per, (cap, per)
    in_maps = []
    for c in range(n_cores):
        shard = np.full(cap, PAD_BIN, dtype=np.int32)
        shard[:per] = flat[c * per:(c + 1) * per]
        in_maps.append({"edges": shard.reshape(tiles * P, cols)})
    return in_maps


def get_nc():
    global _CACHED_NC
    if _CACHED_NC is None:
        _CACHED_NC = build_kernel()
    return _CACHED_NC


def kernel(edge_index: np.ndarray, num_nodes: int = NUM_NODES) -> np.ndarray:
    assert int(num_nodes) == NUM_NODES, "kernel is specialized to 1M nodes"
    edge_index = np.asarray(edge_index)
    assert edge_index.shape == (2, NUM_EDGES), edge_index.shape

    nc = get_nc()
    in_maps = shard_inputs(edge_index)
    res = run_bass_kernel_spmd(nc, in_maps, list(range(N_CORES)))
    out = np.asarray(res.results[0]["out"], dtype=np.float32)
    return out.reshape(-1)[:NUM_NODES]
